# revision 28
# baseline (speedup 1.0000x reference)
"""Causal multi-head attention with RoPE on 8 TRN2 NeuronCores.

Problem: BS=2, SEQ=2048, DIM=2048, NH=16, HD=128 (fp32).
Sharding: core = b*4 + g  (b = batch, g = head-group of 4 heads).
Each core computes q/k/v for its 4 heads from its batch's x, applies RoPE,
causal attention, and a partial output projection through its 512-row slice
of wo. The host sums the 4 per-group partials per batch.

v2 layout (all matmul operands fp16, PSUM accumulation fp32):
  Sweep 1: K+Q together per 512-s-block (8 PSUM banks), one x pass.
  Sweep 2: V per 512-s-block (4 banks, double-buffered).
  Attention: scoresT [j, i] per 128-j-tile with exact causal widths
  (nw = 512-128r on the diagonal), exp on ACT -> f16, shared [128,128]
  triangular mask multiplied in-place on the diagonal 128 columns,
  PV + ones-rowsum accumulation, per-head reciprocal + normalize read
  directly from PSUM, wo deferred by one i-block, f16 output partials
  (host sums in fp32).
Weights prefetch on the scalar-engine DMA queue; x/trig stream on the
sync-engine queue.
"""
import math
import numpy as np
from contextlib import ExitStack

import concourse.bass as bass
import concourse.bacc as bacc
import concourse.tile as tile
import concourse.mybir as mybir
from concourse import bass_utils

F32 = mybir.dt.float32
F16 = mybir.dt.float16
BF16 = mybir.dt.bfloat16
AF = mybir.ActivationFunctionType

SEQ = 2048
DIM = 2048
HD = 128
MG = 512                       # per-core head width (4 heads x 128)
ND = DIM // 128                # 16 d-tiles
NSB = SEQ // 512               # 4 s-blocks
PAIR_SWAP = [i ^ 1 for i in range(32)]

MM_DT = BF16                   # matmul operand dtype
RP_DT = F16                    # rope/trig intermediate dtype (2x DVE rate)

_CACHED = {}


def build_nc(seq=SEQ, mm_dt=None):
    mm_dt = mm_dt or MM_DT
    nsb = seq // 512
    nst = seq // 128
    nc = bacc.Bacc("TRN2", target_bir_lowering=False, debug=False)

    x_d = nc.dram_tensor("x_t", [nsb, 128, ND, 512], mm_dt, kind="ExternalInput")
    wq_d = nc.dram_tensor("wq_t", [128, ND, 512], mm_dt, kind="ExternalInput")
    wk_d = nc.dram_tensor("wk_t", [128, ND, 512], mm_dt, kind="ExternalInput")
    wv_d = nc.dram_tensor("wv_t", [128, ND, 512], mm_dt, kind="ExternalInput")
    wo_d = nc.dram_tensor("wo_t", [128, 4, DIM], mm_dt, kind="ExternalInput")
    trq_d = nc.dram_tensor("trigq", [128, 2, seq], RP_DT, kind="ExternalInput")
    trk_d = nc.dram_tensor("trigk", [128, 2, seq], RP_DT, kind="ExternalInput")
    tri_d = nc.dram_tensor("tri_mask", [128, 128], mm_dt, kind="ExternalInput")
    onc_d = nc.dram_tensor("ones_sq", [128, 128], mm_dt, kind="ExternalInput")
    out_d = nc.dram_tensor("out", [seq, DIM], F16, kind="ExternalOutput")

    with tile.TileContext(nc) as tc, ExitStack() as ctx:
        persist = ctx.enter_context(tc.tile_pool(name="persist", bufs=1))
        ktr = [persist.tile([128, seq], mm_dt, tag=f"ktr{h}", name=f"ktr{h}")
               for h in range(4)]
        qtr = [persist.tile([128, seq], mm_dt, tag=f"qtr{h}", name=f"qtr{h}")
               for h in range(4)]
        v_sb = persist.tile([128, nst, MG], mm_dt, tag="v")
        wk_sb = persist.tile([128, ND, MG], mm_dt, tag="wk")
        wq_sb = persist.tile([128, ND, MG], mm_dt, tag="wq")
        wv_sb = persist.tile([128, ND, MG], mm_dt, tag="wv")
        wo_sb = persist.tile([128, 4, DIM], mm_dt, tag="wo")
        ones_sq = persist.tile([128, 128], mm_dt, tag="onesq")
        tri = persist.tile([128, 128], mm_dt, tag="tri")

        # --- prefetch: x/trig on sync queue (x block 0 first), weights on
        # the scalar queue, small constants on the vector queue ---
        nc.gpsimd.dma_start(ones_sq[:], onc_d.ap())
        nc.gpsimd.dma_start(tri[:], tri_d.ap())
        # first d-tile lands first so the first matmul can start asap
        nc.scalar.dma_start(wk_sb[:, 0:1, :], wk_d.ap()[:, 0:1, :])
        nc.scalar.dma_start(wq_sb[:, 0:1, :], wq_d.ap()[:, 0:1, :])
        nc.scalar.dma_start(wk_sb[:, 1:4, :], wk_d.ap()[:, 1:4, :])
        nc.scalar.dma_start(wq_sb[:, 1:4, :], wq_d.ap()[:, 1:4, :])
        for q4 in range(1, 4):
            nc.scalar.dma_start(wk_sb[:, q4 * 4:(q4 + 1) * 4, :],
                                wk_d.ap()[:, q4 * 4:(q4 + 1) * 4, :])
            nc.scalar.dma_start(wq_sb[:, q4 * 4:(q4 + 1) * 4, :],
                                wq_d.ap()[:, q4 * 4:(q4 + 1) * 4, :])
        for q4 in range(4):
            nc.scalar.dma_start(wv_sb[:, q4 * 4:(q4 + 1) * 4, :],
                                wv_d.ap()[:, q4 * 4:(q4 + 1) * 4, :])
        for c in range(4):
            nc.scalar.dma_start(wo_sb[:, c, :], wo_d.ap()[:, c, :])

        def rope(ropet, prawp, psum_t, out_slice, trig_t):
            """out = p*cos + shuffle(p)*sin; ACT-copy psum->sbuf first so the
            PSUM bank frees without waiting the DVE chain. f16 intermediates
            run the DVE chain at 2x rate."""
            praw = prawp.tile([128, 512], RP_DT, tag="praw", name="praw")
            nc.scalar.copy(praw[:], psum_t[:])
            shuf = ropet.tile([128, 512], RP_DT, tag="shuf", name="shuf")
            nc.vector.stream_shuffle(shuf[:], praw[:], PAIR_SWAP)
            t1 = ropet.tile([128, 512], RP_DT, tag="t1", name="t1")
            nc.vector.tensor_mul(t1[:], praw[:], trig_t[:, 0, :])
            nc.vector.tensor_mul(shuf[:], shuf[:], trig_t[:, 1, :])
            nc.vector.tensor_add(out_slice, t1[:], shuf[:])

        shared = ExitStack()
        trigp = shared.enter_context(tc.tile_pool(name="trigp", bufs=2))
        prawp = shared.enter_context(tc.tile_pool(name="prawp", bufs=4))
        ropet = shared.enter_context(tc.tile_pool(name="ropep", bufs=1))
        xpool = shared.enter_context(tc.tile_pool(name="xpool", bufs=3))

        # ---- sweeps: K + Q in one x pass (8 psum banks), then V reusing
        # the same psum slots (alternating halves) so the transition only
        # waits on the matching rope drains ----
        with tc.tile_pool(name="p0ps", bufs=1, space="PSUM") as p0ps:
            for sb in range(nsb):
                s0 = sb * 512
                xh = []
                for q4 in range(4):
                    xt = xpool.tile([128, 4, 512], mm_dt, tag="x", name="xt")
                    if sb == 0 and q4 == 0:
                        # d-tile chunks on two queues pipeline with the
                        # first d-loop steps
                        for dt0 in range(4):
                            eng = nc.sync if dt0 % 2 == 0 else nc.gpsimd
                            eng.dma_start(xt[:, dt0:dt0 + 1, :],
                                          x_d.ap()[sb, :, dt0:dt0 + 1, :])
                    elif sb == 0 and q4 == 1:
                        nc.sync.dma_start(xt[:, 0:2, :], x_d.ap()[sb, :, 4:6, :])
                        nc.gpsimd.dma_start(xt[:, 2:4, :],
                                            x_d.ap()[sb, :, 6:8, :])
                    else:
                        nc.sync.dma_start(xt[:],
                                          x_d.ap()[sb, :, q4 * 4:q4 * 4 + 4, :])
                    xh.append(xt)
                tgk = trigp.tile([128, 2, 512], RP_DT, tag="tgk", name="tgk")
                nc.sync.dma_start(tgk[:], trk_d.ap()[:, :, s0:s0 + 512])
                tgq = trigp.tile([128, 2, 512], RP_DT, tag="tgq", name="tgq")
                nc.sync.dma_start(tgq[:], trq_d.ap()[:, :, s0:s0 + 512])
                ps_k = [p0ps.tile([128, 512], F32, tag=f"psk{m}", name=f"psk{m}")
                        for m in range(4)]
                ps_q = [p0ps.tile([128, 512], F32, tag=f"psq{m}", name=f"psq{m}")
                        for m in range(4)]
                for dt in range(ND):
                    xs = xh[dt // 4][:, dt % 4, :]
                    for m in range(4):
                        nc.tensor.matmul(
                            ps_k[m][:], wk_sb[:, dt, m * 128:(m + 1) * 128],
                            xs, start=(dt == 0), stop=(dt == ND - 1))
                    for m in range(4):
                        nc.tensor.matmul(
                            ps_q[m][:], wq_sb[:, dt, m * 128:(m + 1) * 128],
                            xs, start=(dt == 0), stop=(dt == ND - 1))
                for m in range(4):
                    rope(ropet, prawp, ps_k[m], ktr[m][:, s0:s0 + 512], tgk)
                for m in range(4):
                    rope(ropet, prawp, ps_q[m], qtr[m][:, s0:s0 + 512], tgq)

            # ---- V sweep inside the same pool: block sb uses the psk
            # slots (even sb) / psq slots (odd sb) for double buffering ----
            for sb in range(nsb):
                grp = "psk" if sb % 2 == 0 else "psq"
                ps_v = [p0ps.tile([128, 512], F32, tag=f"{grp}{st}",
                                  name=f"psv{st}")
                        for st in range(4)]
                xh = []
                for q4 in range(4):
                    xt = xpool.tile([128, 4, 512], mm_dt, tag="x", name="xt")
                    nc.sync.dma_start(xt[:], x_d.ap()[sb, :, q4 * 4:q4 * 4 + 4, :])
                    xh.append(xt)
                for dt in range(ND):
                    xs = xh[dt // 4][:, dt % 4, :]
                    for st in range(4):
                        nc.tensor.matmul(
                            ps_v[st][:], xs[:, st * 128:(st + 1) * 128],
                            wv_sb[:, dt, :],
                            start=(dt == 0), stop=(dt == ND - 1))
                for st in range(4):
                    if st % 2 == 0:
                        nc.scalar.copy(v_sb[:, sb * 4 + st, :], ps_v[st][:])
                    else:
                        nc.vector.tensor_copy(v_sb[:, sb * 4 + st, :],
                                              ps_v[st][:])

        shared.close()

        # ---- attention + wo ----
        with tc.tile_pool(name="ep", bufs=8) as ep, \
             tc.tile_pool(name="otn", bufs=2) as otn, \
             tc.tile_pool(name="bcp", bufs=2) as bcp, \
             tc.tile_pool(name="wout", bufs=4) as wout, \
             tc.tile_pool(name="ps_s", bufs=3, space="PSUM") as ps_s, \
             tc.tile_pool(name="ps_o", bufs=2, space="PSUM") as ps_o, \
             tc.tile_pool(name="ps_r", bufs=1, space="PSUM") as ps_r, \
             tc.tile_pool(name="ps_w", bufs=2, space="PSUM") as ps_w:

            def wo_block(it):
                for dblk in range(4):
                    pw = ps_w.tile([128, 512], F32, tag="w", name="w")
                    for c in range(4):
                        nc.tensor.matmul(
                            pw[:], prev[0][:, c, it * 128:(it + 1) * 128],
                            wo_sb[:, c, dblk * 512:(dblk + 1) * 512],
                            start=(c == 0), stop=(c == 3))
                    ow = wout.tile([128, 512], F16, tag="ow", name="ow")
                    if dblk % 2 == 0:
                        nc.scalar.copy(ow[:], pw[:])
                    else:
                        nc.vector.tensor_copy(ow[:], pw[:])
                    eng = (nc.sync, nc.scalar)[dblk % 2]
                    eng.dma_start(
                        out_d.ap()[prev[1] + it * 128:prev[1] + (it + 1) * 128,
                                   dblk * 512:(dblk + 1) * 512], ow[:])

            prev = None
            for ib in range(nsb):
                i0 = ib * 512
                nj = 4 * ib + 4
                outn = otn.tile([128, 4, 512], mm_dt, tag="outn", name="outn")
                rsum = bcp.tile([128, 4, 512], F32, tag="rsum", name="rsum")
                rbc = bcp.tile([128, 4, 512], F32, tag="rbc", name="rbc")
                for h in range(4):
                    po = ps_o.tile([128, 512], F32, tag="pv", name="pv")
                    prbc = ps_r.tile([128, 512], F32, tag="rs", name="rs")
                    for tj in range(nj):
                        r = tj - 4 * ib
                        i_lo = 128 * r if r >= 0 else 0
                        pscr = ps_s.tile([128, 512], F32, tag="sc", name="sc")
                        nc.tensor.matmul(
                            pscr[:, i_lo:512], ktr[h][:, tj * 128:(tj + 1) * 128],
                            qtr[h][:, i0 + i_lo:i0 + 512],
                            start=True, stop=True)
                        e_t = ep.tile([128, 512], mm_dt, tag="e", name="e")
                        nc.scalar.activation(e_t[:, i_lo:512], pscr[:, i_lo:512],
                                             AF.Exp)
                        if r >= 0:                # diagonal tile: mask 128 cols
                            nc.vector.tensor_mul(
                                e_t[:, i_lo:i_lo + 128],
                                e_t[:, i_lo:i_lo + 128], tri[:])
                        nc.tensor.matmul(
                            po[:, i_lo:512],
                            v_sb[:, tj, h * 128:(h + 1) * 128], e_t[:, i_lo:512],
                            start=(tj == 0), stop=(tj == nj - 1))
                        nc.tensor.matmul(
                            prbc[:, i_lo:512], ones_sq[:], e_t[:, i_lo:512],
                            start=(tj == 0), stop=(tj == nj - 1))
                    # fast ACT drain frees the single ps_r bank; the slow DVE
                    # reciprocal then runs off the PE-critical path
                    nc.scalar.copy(rsum[:, h, :], prbc[:])
                    nc.vector.reciprocal_approx_fast(rbc[:, h, :], rsum[:, h, :])
                    nc.vector.tensor_mul(outn[:, h, :], po[:], rbc[:, h, :])
                    if prev is not None:
                        wo_block(h)
                prev = (outn, i0)

            for it in range(4):
                wo_block(it)

    nc.compile()
    return nc


def _host_prep(x, freqs_cos, freqs_sin, wq, wk, wv, wo, mm_dt=None, seq=SEQ):
    """Build the 8 per-core input maps with pre-tiled layouts."""
    mm_dt = mm_dt or MM_DT
    npdt = mybir.dt.np(mm_dt)
    bs = x.shape[0]
    nsb = seq // 512
    scale = np.float32(1.0 / math.sqrt(HD))

    rp_npdt = mybir.dt.np(RP_DT)
    cos_e = np.repeat(np.asarray(freqs_cos).T, 2, axis=0).astype(np.float32)
    sin_raw = np.repeat(np.asarray(freqs_sin).T, 2, axis=0).astype(np.float32)
    sin_e = sin_raw.copy()
    sin_e[0::2] = -sin_raw[0::2]      # out[2i] = q[2i]cos - q[2i+1]sin
    trigk = np.ascontiguousarray(np.stack([cos_e, sin_e], axis=1)).astype(rp_npdt)
    trigq = np.ascontiguousarray(
        np.stack([cos_e, sin_e], axis=1) * scale).astype(rp_npdt)

    jr = np.arange(128)[:, None]
    cr = np.arange(128)[None, :]
    tri = (jr <= cr).astype(npdt)     # [j, c]: keep c >= j

    ones_sq = np.ones((128, 128), npdt)

    def wtile(w):  # [DIM, 512] -> [128, 16, 512]
        return np.ascontiguousarray(
            np.asarray(w).reshape(ND, 128, MG).transpose(1, 0, 2)).astype(npdt)

    x_t = []
    for b in range(bs):
        xt = np.asarray(x[b]).reshape(nsb, 512, ND, 128).transpose(0, 3, 2, 1)
        x_t.append(np.ascontiguousarray(xt).astype(npdt))

    in_maps = []
    for core in range(8):
        b, g = divmod(core, 4)
        b = min(b, bs - 1)
        wo_g = np.asarray(wo)[g * MG:(g + 1) * MG, :]
        in_maps.append({
            "x_t": x_t[b],
            "wq_t": wtile(np.asarray(wq)[:, g * MG:(g + 1) * MG]),
            "wk_t": wtile(np.asarray(wk)[:, g * MG:(g + 1) * MG]),
            "wv_t": wtile(np.asarray(wv)[:, g * MG:(g + 1) * MG]),
            "wo_t": np.ascontiguousarray(
                wo_g.reshape(4, 128, DIM).transpose(1, 0, 2)).astype(npdt),
            "trigq": trigq, "trigk": trigk,
            "tri_mask": tri, "ones_sq": ones_sq,
        })
    return in_maps


def kernel(x, freqs_cos, freqs_sin, mask, wq, wk, wv, wo, _trace=False):
    x = np.asarray(x, dtype=np.float32)
    in_maps = _host_prep(x, np.asarray(freqs_cos), np.asarray(freqs_sin),
                         np.asarray(wq), np.asarray(wk), np.asarray(wv),
                         np.asarray(wo))
    if "nc" not in _CACHED:
        _CACHED["nc"] = build_nc()
    nc = _CACHED["nc"]
    res = bass_utils.run_bass_kernel_spmd(nc, in_maps, core_ids=list(range(8)),
                                          trace=_trace)
    if _trace:
        _CACHED["last_exec_time_ns"] = res.exec_time_ns
        _CACHED["last_trace"] = res.instructions_and_trace
    bs = x.shape[0]
    out = np.zeros((bs, SEQ, DIM), dtype=np.float32)
    for core in range(8):
        out[core // 4] += res.results[core]["out"].astype(np.float32)
    return out


# revision 29
# speedup vs baseline: 1.1878x; 1.1878x over previous
"""Causal multi-head attention with RoPE on 8 TRN2 NeuronCores.

Problem: BS=2, SEQ=2048, DIM=2048, NH=16, HD=128 (fp32).
Sharding: core = b*4 + g  (b = batch, g = head-group of 4 heads).
Each core computes q/k/v for its 4 heads from its batch's x, applies RoPE,
causal attention, and a partial output projection through its 512-row slice
of wo. The host sums the 4 per-group partials per batch.

v2 layout (all matmul operands fp16, PSUM accumulation fp32):
  Sweep 1: K+Q together per 512-s-block (8 PSUM banks), one x pass.
  Sweep 2: V per 512-s-block (4 banks, double-buffered).
  Attention: scoresT [j, i] per 128-j-tile with exact causal widths
  (nw = 512-128r on the diagonal), exp on ACT -> f16, shared [128,128]
  triangular mask multiplied in-place on the diagonal 128 columns,
  PV + ones-rowsum accumulation, per-head reciprocal + normalize read
  directly from PSUM, wo deferred by one i-block, f16 output partials
  (host sums in fp32).
Weights prefetch on the scalar-engine DMA queue; x/trig stream on the
sync-engine queue.
"""
import math
import numpy as np
from contextlib import ExitStack

import concourse.bass as bass
import concourse.bacc as bacc
import concourse.tile as tile
import concourse.mybir as mybir
from concourse import bass_utils

F32 = mybir.dt.float32
F16 = mybir.dt.float16
BF16 = mybir.dt.bfloat16
AF = mybir.ActivationFunctionType

SEQ = 2048
DIM = 2048
HD = 128
MG = 512                       # per-core head width (4 heads x 128)
ND = DIM // 128                # 16 d-tiles
NSB = SEQ // 512               # 4 s-blocks
PAIR_SWAP = [i ^ 1 for i in range(32)]

MM_DT = BF16                   # matmul operand dtype
RP_DT = F16                    # rope/trig intermediate dtype (2x DVE rate)

_CACHED = {}


def build_nc(seq=SEQ, mm_dt=None):
    mm_dt = mm_dt or MM_DT
    nsb = seq // 512
    nst = seq // 128
    nc = bacc.Bacc("TRN2", target_bir_lowering=False, debug=False)

    x_d = nc.dram_tensor("x_t", [nsb, 128, ND, 512], mm_dt, kind="ExternalInput")
    wq_d = nc.dram_tensor("wq_t", [128, ND, 512], mm_dt, kind="ExternalInput")
    wk_d = nc.dram_tensor("wk_t", [128, ND, 512], mm_dt, kind="ExternalInput")
    wv_d = nc.dram_tensor("wv_t", [128, ND, 512], mm_dt, kind="ExternalInput")
    wo_d = nc.dram_tensor("wo_t", [128, 4, DIM], mm_dt, kind="ExternalInput")
    trq_d = nc.dram_tensor("trigq", [128, 2, seq], RP_DT, kind="ExternalInput")
    trk_d = nc.dram_tensor("trigk", [128, 2, seq], RP_DT, kind="ExternalInput")
    tri_d = nc.dram_tensor("tri_mask", [128, 128], mm_dt, kind="ExternalInput")
    onc_d = nc.dram_tensor("ones_sq", [128, 128], mm_dt, kind="ExternalInput")
    out_d = nc.dram_tensor("out", [seq, DIM], F16, kind="ExternalOutput")

    with tile.TileContext(nc) as tc, ExitStack() as ctx:
        persist = ctx.enter_context(tc.tile_pool(name="persist", bufs=1))
        ktr = [persist.tile([128, seq], mm_dt, tag=f"ktr{h}", name=f"ktr{h}")
               for h in range(4)]
        qtr = [persist.tile([128, seq], mm_dt, tag=f"qtr{h}", name=f"qtr{h}")
               for h in range(4)]
        v_sb = persist.tile([128, nst, MG], mm_dt, tag="v")
        wk_sb = persist.tile([128, ND, MG], mm_dt, tag="wk")
        wq_sb = persist.tile([128, ND, MG], mm_dt, tag="wq")
        wv_sb = persist.tile([128, ND, MG], mm_dt, tag="wv")
        wo_sb = persist.tile([128, 4, DIM], mm_dt, tag="wo")
        ones_sq = persist.tile([128, 128], mm_dt, tag="onesq")
        tri = persist.tile([128, 128], mm_dt, tag="tri")

        # --- prefetch: x/trig on sync queue (x block 0 first), weights on
        # the scalar queue, small constants on the vector queue ---
        nc.gpsimd.dma_start(ones_sq[:], onc_d.ap())
        nc.gpsimd.dma_start(tri[:], tri_d.ap())
        # first d-tile lands first so the first matmul can start asap
        nc.scalar.dma_start(wk_sb[:, 0:1, :], wk_d.ap()[:, 0:1, :])
        nc.scalar.dma_start(wq_sb[:, 0:1, :], wq_d.ap()[:, 0:1, :])
        nc.scalar.dma_start(wk_sb[:, 1:4, :], wk_d.ap()[:, 1:4, :])
        nc.scalar.dma_start(wq_sb[:, 1:4, :], wq_d.ap()[:, 1:4, :])
        for q4 in range(1, 4):
            nc.scalar.dma_start(wk_sb[:, q4 * 4:(q4 + 1) * 4, :],
                                wk_d.ap()[:, q4 * 4:(q4 + 1) * 4, :])
            nc.scalar.dma_start(wq_sb[:, q4 * 4:(q4 + 1) * 4, :],
                                wq_d.ap()[:, q4 * 4:(q4 + 1) * 4, :])
        for q4 in range(4):
            nc.scalar.dma_start(wv_sb[:, q4 * 4:(q4 + 1) * 4, :],
                                wv_d.ap()[:, q4 * 4:(q4 + 1) * 4, :])
        for c in range(4):
            nc.scalar.dma_start(wo_sb[:, c, :], wo_d.ap()[:, c, :])

        def rope(ropet, prawp, psum_t, out_slice, trig_t):
            """out = p*cos + shuffle(p)*sin; ACT-copy psum->sbuf first so the
            PSUM bank frees without waiting the DVE chain. f16 intermediates
            run the DVE chain at 2x rate."""
            praw = prawp.tile([128, 512], RP_DT, tag="praw", name="praw")
            nc.scalar.copy(praw[:], psum_t[:])
            shuf = ropet.tile([128, 512], RP_DT, tag="shuf", name="shuf")
            nc.vector.stream_shuffle(shuf[:], praw[:], PAIR_SWAP)
            t1 = ropet.tile([128, 512], RP_DT, tag="t1", name="t1")
            nc.vector.tensor_mul(t1[:], praw[:], trig_t[:, 0, :])
            nc.vector.tensor_mul(shuf[:], shuf[:], trig_t[:, 1, :])
            nc.vector.tensor_add(out_slice, t1[:], shuf[:])

        shared = ExitStack()
        trigp = shared.enter_context(tc.tile_pool(name="trigp", bufs=2))
        prawp = shared.enter_context(tc.tile_pool(name="prawp", bufs=4))
        ropet = shared.enter_context(tc.tile_pool(name="ropep", bufs=1))
        xpool = shared.enter_context(tc.tile_pool(name="xpool", bufs=3))

        # ---- sweeps: K + Q in one x pass (8 psum banks), then V reusing
        # the same psum slots (alternating halves) so the transition only
        # waits on the matching rope drains ----
        with tc.tile_pool(name="p0ps", bufs=1, space="PSUM") as p0ps:
            for sb in range(nsb):
                s0 = sb * 512
                xh = []
                for q4 in range(4):
                    xt = xpool.tile([128, 4, 512], mm_dt, tag="x", name="xt")
                    if sb == 0 and q4 == 0:
                        # d-tile chunks pipeline with the first d-loop steps
                        for dt0 in range(4):
                            nc.sync.dma_start(xt[:, dt0:dt0 + 1, :],
                                              x_d.ap()[sb, :, dt0:dt0 + 1, :])
                    elif sb == 0 and q4 == 1:
                        nc.sync.dma_start(xt[:, 0:2, :], x_d.ap()[sb, :, 4:6, :])
                        nc.sync.dma_start(xt[:, 2:4, :], x_d.ap()[sb, :, 6:8, :])
                    else:
                        nc.sync.dma_start(xt[:],
                                          x_d.ap()[sb, :, q4 * 4:q4 * 4 + 4, :])
                    xh.append(xt)
                tgk = trigp.tile([128, 2, 512], RP_DT, tag="tgk", name="tgk")
                nc.sync.dma_start(tgk[:], trk_d.ap()[:, :, s0:s0 + 512])
                tgq = trigp.tile([128, 2, 512], RP_DT, tag="tgq", name="tgq")
                nc.sync.dma_start(tgq[:], trq_d.ap()[:, :, s0:s0 + 512])
                ps_k = [p0ps.tile([128, 512], F32, tag=f"psk{m}", name=f"psk{m}")
                        for m in range(4)]
                ps_q = [p0ps.tile([128, 512], F32, tag=f"psq{m}", name=f"psq{m}")
                        for m in range(4)]
                for dt in range(ND):
                    xs = xh[dt // 4][:, dt % 4, :]
                    for m in range(4):
                        nc.tensor.matmul(
                            ps_k[m][:], wk_sb[:, dt, m * 128:(m + 1) * 128],
                            xs, start=(dt == 0), stop=(dt == ND - 1))
                    for m in range(4):
                        nc.tensor.matmul(
                            ps_q[m][:], wq_sb[:, dt, m * 128:(m + 1) * 128],
                            xs, start=(dt == 0), stop=(dt == ND - 1))
                for m in range(4):
                    rope(ropet, prawp, ps_k[m], ktr[m][:, s0:s0 + 512], tgk)
                for m in range(4):
                    rope(ropet, prawp, ps_q[m], qtr[m][:, s0:s0 + 512], tgq)

            # ---- V sweep inside the same pool: block sb uses the psk
            # slots (even sb) / psq slots (odd sb) for double buffering ----
            for sb in range(nsb):
                grp = "psk" if sb % 2 == 0 else "psq"
                ps_v = [p0ps.tile([128, 512], F32, tag=f"{grp}{st}",
                                  name=f"psv{st}")
                        for st in range(4)]
                xh = []
                for q4 in range(4):
                    xt = xpool.tile([128, 4, 512], mm_dt, tag="x", name="xt")
                    nc.sync.dma_start(xt[:], x_d.ap()[sb, :, q4 * 4:q4 * 4 + 4, :])
                    xh.append(xt)
                for dt in range(ND):
                    xs = xh[dt // 4][:, dt % 4, :]
                    for st in range(4):
                        nc.tensor.matmul(
                            ps_v[st][:], xs[:, st * 128:(st + 1) * 128],
                            wv_sb[:, dt, :],
                            start=(dt == 0), stop=(dt == ND - 1))
                for st in range(4):
                    if st % 2 == 0:
                        nc.scalar.copy(v_sb[:, sb * 4 + st, :], ps_v[st][:])
                    else:
                        nc.vector.tensor_copy(v_sb[:, sb * 4 + st, :],
                                              ps_v[st][:])

        shared.close()

        # ---- attention + wo ----
        with tc.tile_pool(name="ep", bufs=8) as ep, \
             tc.tile_pool(name="otn", bufs=2) as otn, \
             tc.tile_pool(name="bcp", bufs=2) as bcp, \
             tc.tile_pool(name="wout", bufs=4) as wout, \
             tc.tile_pool(name="ps_s", bufs=3, space="PSUM") as ps_s, \
             tc.tile_pool(name="ps_o", bufs=2, space="PSUM") as ps_o, \
             tc.tile_pool(name="ps_r", bufs=1, space="PSUM") as ps_r, \
             tc.tile_pool(name="ps_w", bufs=2, space="PSUM") as ps_w:

            def wo_block(it):
                for dblk in range(4):
                    pw = ps_w.tile([128, 512], F32, tag="w", name="w")
                    for c in range(4):
                        nc.tensor.matmul(
                            pw[:], prev[0][:, c, it * 128:(it + 1) * 128],
                            wo_sb[:, c, dblk * 512:(dblk + 1) * 512],
                            start=(c == 0), stop=(c == 3))
                    ow = wout.tile([128, 512], F16, tag="ow", name="ow")
                    if dblk % 2 == 0:
                        nc.scalar.copy(ow[:], pw[:])
                    else:
                        nc.vector.tensor_copy(ow[:], pw[:])
                    eng = (nc.sync, nc.scalar)[dblk % 2]
                    eng.dma_start(
                        out_d.ap()[prev[1] + it * 128:prev[1] + (it + 1) * 128,
                                   dblk * 512:(dblk + 1) * 512], ow[:])

            prev = None
            for ib in range(nsb):
                i0 = ib * 512
                nj = 4 * ib + 4
                outn = otn.tile([128, 4, 512], mm_dt, tag="outn", name="outn")
                rsum = bcp.tile([128, 4, 512], F32, tag="rsum", name="rsum")
                rbc = bcp.tile([128, 4, 512], F32, tag="rbc", name="rbc")
                for h in range(4):
                    po = ps_o.tile([128, 512], F32, tag="pv", name="pv")
                    prbc = ps_r.tile([128, 512], F32, tag="rs", name="rs")
                    for tj in range(nj):
                        r = tj - 4 * ib
                        i_lo = 128 * r if r >= 0 else 0
                        pscr = ps_s.tile([128, 512], F32, tag="sc", name="sc")
                        nc.tensor.matmul(
                            pscr[:, i_lo:512], ktr[h][:, tj * 128:(tj + 1) * 128],
                            qtr[h][:, i0 + i_lo:i0 + 512],
                            start=True, stop=True)
                        e_t = ep.tile([128, 512], mm_dt, tag="e", name="e")
                        nc.scalar.activation(e_t[:, i_lo:512], pscr[:, i_lo:512],
                                             AF.Exp)
                        if r >= 0:                # diagonal tile: mask 128 cols
                            nc.vector.tensor_mul(
                                e_t[:, i_lo:i_lo + 128],
                                e_t[:, i_lo:i_lo + 128], tri[:])
                        nc.tensor.matmul(
                            po[:, i_lo:512],
                            v_sb[:, tj, h * 128:(h + 1) * 128], e_t[:, i_lo:512],
                            start=(tj == 0), stop=(tj == nj - 1))
                        nc.tensor.matmul(
                            prbc[:, i_lo:512], ones_sq[:], e_t[:, i_lo:512],
                            start=(tj == 0), stop=(tj == nj - 1))
                    # fast ACT drain frees the single ps_r bank; the slow DVE
                    # reciprocal then runs off the PE-critical path
                    nc.scalar.copy(rsum[:, h, :], prbc[:])
                    nc.vector.reciprocal_approx_fast(rbc[:, h, :], rsum[:, h, :])
                    nc.vector.tensor_mul(outn[:, h, :], po[:], rbc[:, h, :])
                    if prev is not None:
                        wo_block(h)
                prev = (outn, i0)

            for it in range(4):
                wo_block(it)

    nc.compile()
    return nc


def _host_prep(x, freqs_cos, freqs_sin, wq, wk, wv, wo, mm_dt=None, seq=SEQ):
    """Build the 8 per-core input maps with pre-tiled layouts."""
    mm_dt = mm_dt or MM_DT
    npdt = mybir.dt.np(mm_dt)
    bs = x.shape[0]
    nsb = seq // 512
    scale = np.float32(1.0 / math.sqrt(HD))

    rp_npdt = mybir.dt.np(RP_DT)
    cos_e = np.repeat(np.asarray(freqs_cos).T, 2, axis=0).astype(np.float32)
    sin_raw = np.repeat(np.asarray(freqs_sin).T, 2, axis=0).astype(np.float32)
    sin_e = sin_raw.copy()
    sin_e[0::2] = -sin_raw[0::2]      # out[2i] = q[2i]cos - q[2i+1]sin
    trigk = np.ascontiguousarray(np.stack([cos_e, sin_e], axis=1)).astype(rp_npdt)
    trigq = np.ascontiguousarray(
        np.stack([cos_e, sin_e], axis=1) * scale).astype(rp_npdt)

    jr = np.arange(128)[:, None]
    cr = np.arange(128)[None, :]
    tri = (jr <= cr).astype(npdt)     # [j, c]: keep c >= j

    ones_sq = np.ones((128, 128), npdt)

    def wtile(w):  # [DIM, 512] -> [128, 16, 512]
        return np.ascontiguousarray(
            np.asarray(w).reshape(ND, 128, MG).transpose(1, 0, 2)).astype(npdt)

    x_t = []
    for b in range(bs):
        xt = np.asarray(x[b]).reshape(nsb, 512, ND, 128).transpose(0, 3, 2, 1)
        x_t.append(np.ascontiguousarray(xt).astype(npdt))

    in_maps = []
    for core in range(8):
        b, g = divmod(core, 4)
        b = min(b, bs - 1)
        wo_g = np.asarray(wo)[g * MG:(g + 1) * MG, :]
        in_maps.append({
            "x_t": x_t[b],
            "wq_t": wtile(np.asarray(wq)[:, g * MG:(g + 1) * MG]),
            "wk_t": wtile(np.asarray(wk)[:, g * MG:(g + 1) * MG]),
            "wv_t": wtile(np.asarray(wv)[:, g * MG:(g + 1) * MG]),
            "wo_t": np.ascontiguousarray(
                wo_g.reshape(4, 128, DIM).transpose(1, 0, 2)).astype(npdt),
            "trigq": trigq, "trigk": trigk,
            "tri_mask": tri, "ones_sq": ones_sq,
        })
    return in_maps


def kernel(x, freqs_cos, freqs_sin, mask, wq, wk, wv, wo, _trace=False):
    x = np.asarray(x, dtype=np.float32)
    in_maps = _host_prep(x, np.asarray(freqs_cos), np.asarray(freqs_sin),
                         np.asarray(wq), np.asarray(wk), np.asarray(wv),
                         np.asarray(wo))
    if "nc" not in _CACHED:
        _CACHED["nc"] = build_nc()
    nc = _CACHED["nc"]
    res = bass_utils.run_bass_kernel_spmd(nc, in_maps, core_ids=list(range(8)),
                                          trace=_trace)
    if _trace:
        _CACHED["last_exec_time_ns"] = res.exec_time_ns
        _CACHED["last_trace"] = res.instructions_and_trace
    bs = x.shape[0]
    out = np.zeros((bs, SEQ, DIM), dtype=np.float32)
    for core in range(8):
        out[core // 4] += res.results[core]["out"].astype(np.float32)
    return out


# revision 30
# speedup vs baseline: 1.2056x; 1.0149x over previous
"""Causal multi-head attention with RoPE on 8 TRN2 NeuronCores.

Problem: BS=2, SEQ=2048, DIM=2048, NH=16, HD=128 (fp32).
Sharding: core = b*4 + g  (b = batch, g = head-group of 4 heads).
Each core computes q/k/v for its 4 heads from its batch's x, applies RoPE,
causal attention, and a partial output projection through its 512-row slice
of wo. The host sums the 4 per-group partials per batch.

v2 layout (all matmul operands fp16, PSUM accumulation fp32):
  Sweep 1: K+Q together per 512-s-block (8 PSUM banks), one x pass.
  Sweep 2: V per 512-s-block (4 banks, double-buffered).
  Attention: scoresT [j, i] per 128-j-tile with exact causal widths
  (nw = 512-128r on the diagonal), exp on ACT -> f16, shared [128,128]
  triangular mask multiplied in-place on the diagonal 128 columns,
  PV + ones-rowsum accumulation, per-head reciprocal + normalize read
  directly from PSUM, wo deferred by one i-block, f16 output partials
  (host sums in fp32).
Weights prefetch on the scalar-engine DMA queue; x/trig stream on the
sync-engine queue.
"""
import math
import numpy as np
from contextlib import ExitStack

import concourse.bass as bass
import concourse.bacc as bacc
import concourse.tile as tile
import concourse.mybir as mybir
from concourse import bass_utils

F32 = mybir.dt.float32
F16 = mybir.dt.float16
BF16 = mybir.dt.bfloat16
AF = mybir.ActivationFunctionType

SEQ = 2048
DIM = 2048
HD = 128
MG = 512                       # per-core head width (4 heads x 128)
ND = DIM // 128                # 16 d-tiles
NSB = SEQ // 512               # 4 s-blocks
PAIR_SWAP = [i ^ 1 for i in range(32)]

MM_DT = BF16                   # matmul operand dtype
RP_DT = F16                    # rope/trig intermediate dtype (2x DVE rate)

_CACHED = {}


def build_nc(seq=SEQ, mm_dt=None):
    mm_dt = mm_dt or MM_DT
    nsb = seq // 512
    nst = seq // 128
    nc = bacc.Bacc("TRN2", target_bir_lowering=False, debug=False)

    x_d = nc.dram_tensor("x_t", [nsb, 128, ND, 512], mm_dt, kind="ExternalInput")
    wq_d = nc.dram_tensor("wq_t", [128, ND, 512], mm_dt, kind="ExternalInput")
    wk_d = nc.dram_tensor("wk_t", [128, ND, 512], mm_dt, kind="ExternalInput")
    wv_d = nc.dram_tensor("wv_t", [128, ND, 512], mm_dt, kind="ExternalInput")
    wo_d = nc.dram_tensor("wo_t", [128, 4, DIM], mm_dt, kind="ExternalInput")
    trq_d = nc.dram_tensor("trigq", [128, 2, seq], RP_DT, kind="ExternalInput")
    trk_d = nc.dram_tensor("trigk", [128, 2, seq], RP_DT, kind="ExternalInput")
    tri_d = nc.dram_tensor("tri_mask", [128, 128], mm_dt, kind="ExternalInput")
    onc_d = nc.dram_tensor("ones_sq", [128, 128], mm_dt, kind="ExternalInput")
    out_d = nc.dram_tensor("out", [seq, DIM], F16, kind="ExternalOutput")

    with tile.TileContext(nc) as tc, ExitStack() as ctx:
        persist = ctx.enter_context(tc.tile_pool(name="persist", bufs=1))
        ktr = [persist.tile([128, seq], mm_dt, tag=f"ktr{h}", name=f"ktr{h}")
               for h in range(4)]
        qtr = [persist.tile([128, seq], mm_dt, tag=f"qtr{h}", name=f"qtr{h}")
               for h in range(4)]
        v_sb = persist.tile([128, nst, MG], mm_dt, tag="v")
        wk_sb = persist.tile([128, ND, MG], mm_dt, tag="wk")
        wq_sb = persist.tile([128, ND, MG], mm_dt, tag="wq")
        wv_sb = persist.tile([128, ND, MG], mm_dt, tag="wv")
        wo_sb = persist.tile([128, 4, DIM], mm_dt, tag="wo")
        ones_sq = persist.tile([128, 128], mm_dt, tag="onesq")
        tri = persist.tile([128, 128], mm_dt, tag="tri")

        # --- prefetch: x/trig on sync queue (x block 0 first), weights on
        # the scalar queue, small constants on the vector queue ---
        nc.gpsimd.dma_start(ones_sq[:], onc_d.ap())
        nc.gpsimd.dma_start(tri[:], tri_d.ap())
        # first d-tile lands first so the first matmul can start asap
        nc.scalar.dma_start(wk_sb[:, 0:1, :], wk_d.ap()[:, 0:1, :])
        nc.scalar.dma_start(wq_sb[:, 0:1, :], wq_d.ap()[:, 0:1, :])
        nc.scalar.dma_start(wk_sb[:, 1:4, :], wk_d.ap()[:, 1:4, :])
        nc.scalar.dma_start(wq_sb[:, 1:4, :], wq_d.ap()[:, 1:4, :])
        for q4 in range(1, 4):
            nc.scalar.dma_start(wk_sb[:, q4 * 4:(q4 + 1) * 4, :],
                                wk_d.ap()[:, q4 * 4:(q4 + 1) * 4, :])
            nc.scalar.dma_start(wq_sb[:, q4 * 4:(q4 + 1) * 4, :],
                                wq_d.ap()[:, q4 * 4:(q4 + 1) * 4, :])
        for q4 in range(4):
            nc.scalar.dma_start(wv_sb[:, q4 * 4:(q4 + 1) * 4, :],
                                wv_d.ap()[:, q4 * 4:(q4 + 1) * 4, :])
        for c in range(4):
            nc.scalar.dma_start(wo_sb[:, c, :], wo_d.ap()[:, c, :])

        def rope(ropet, prawp, psum_t, out_slice, trig_t):
            """out = p*cos + shuffle(p)*sin; ACT-copy psum->sbuf first so the
            PSUM bank frees without waiting the DVE chain. f16 intermediates
            run the DVE chain at 2x rate."""
            praw = prawp.tile([128, 512], RP_DT, tag="praw", name="praw")
            nc.scalar.copy(praw[:], psum_t[:])
            shuf = ropet.tile([128, 512], RP_DT, tag="shuf", name="shuf")
            nc.vector.stream_shuffle(shuf[:], praw[:], PAIR_SWAP)
            t1 = ropet.tile([128, 512], RP_DT, tag="t1", name="t1")
            nc.vector.tensor_mul(t1[:], praw[:], trig_t[:, 0, :])
            nc.vector.tensor_mul(shuf[:], shuf[:], trig_t[:, 1, :])
            nc.vector.tensor_add(out_slice, t1[:], shuf[:])

        shared = ExitStack()
        trigp = shared.enter_context(tc.tile_pool(name="trigp", bufs=2))
        prawp = shared.enter_context(tc.tile_pool(name="prawp", bufs=6))
        ropet = shared.enter_context(tc.tile_pool(name="ropep", bufs=2))
        xpool = shared.enter_context(tc.tile_pool(name="xpool", bufs=4))

        # ---- sweeps: K + Q in one x pass (8 psum banks), then V reusing
        # the same psum slots (alternating halves) so the transition only
        # waits on the matching rope drains ----
        with tc.tile_pool(name="p0ps", bufs=1, space="PSUM") as p0ps:
            for sb in range(nsb):
                s0 = sb * 512
                xh = []
                for q4 in range(4):
                    xt = xpool.tile([128, 4, 512], mm_dt, tag="x", name="xt")
                    if sb == 0 and q4 == 0:
                        # d-tile chunks pipeline with the first d-loop steps
                        for dt0 in range(4):
                            nc.sync.dma_start(xt[:, dt0:dt0 + 1, :],
                                              x_d.ap()[sb, :, dt0:dt0 + 1, :])
                    elif sb == 0 and q4 == 1:
                        nc.sync.dma_start(xt[:, 0:2, :], x_d.ap()[sb, :, 4:6, :])
                        nc.sync.dma_start(xt[:, 2:4, :], x_d.ap()[sb, :, 6:8, :])
                    else:
                        nc.sync.dma_start(xt[:],
                                          x_d.ap()[sb, :, q4 * 4:q4 * 4 + 4, :])
                    xh.append(xt)
                tgk = trigp.tile([128, 2, 512], RP_DT, tag="tgk", name="tgk")
                nc.sync.dma_start(tgk[:], trk_d.ap()[:, :, s0:s0 + 512])
                tgq = trigp.tile([128, 2, 512], RP_DT, tag="tgq", name="tgq")
                nc.sync.dma_start(tgq[:], trq_d.ap()[:, :, s0:s0 + 512])
                ps_k = [p0ps.tile([128, 512], F32, tag=f"psk{m}", name=f"psk{m}")
                        for m in range(4)]
                ps_q = [p0ps.tile([128, 512], F32, tag=f"psq{m}", name=f"psq{m}")
                        for m in range(4)]
                for dt in range(ND):
                    xs = xh[dt // 4][:, dt % 4, :]
                    for m in range(4):
                        nc.tensor.matmul(
                            ps_k[m][:], wk_sb[:, dt, m * 128:(m + 1) * 128],
                            xs, start=(dt == 0), stop=(dt == ND - 1))
                    for m in range(4):
                        nc.tensor.matmul(
                            ps_q[m][:], wq_sb[:, dt, m * 128:(m + 1) * 128],
                            xs, start=(dt == 0), stop=(dt == ND - 1))
                for m in range(4):
                    rope(ropet, prawp, ps_k[m], ktr[m][:, s0:s0 + 512], tgk)
                for m in range(4):
                    rope(ropet, prawp, ps_q[m], qtr[m][:, s0:s0 + 512], tgq)

            # ---- V sweep inside the same pool: block sb uses the psk
            # slots (even sb) / psq slots (odd sb) for double buffering ----
            for sb in range(nsb):
                grp = "psk" if sb % 2 == 0 else "psq"
                ps_v = [p0ps.tile([128, 512], F32, tag=f"{grp}{st}",
                                  name=f"psv{st}")
                        for st in range(4)]
                xh = []
                for q4 in range(4):
                    xt = xpool.tile([128, 4, 512], mm_dt, tag="x", name="xt")
                    nc.sync.dma_start(xt[:], x_d.ap()[sb, :, q4 * 4:q4 * 4 + 4, :])
                    xh.append(xt)
                for dt in range(ND):
                    xs = xh[dt // 4][:, dt % 4, :]
                    for st in range(4):
                        nc.tensor.matmul(
                            ps_v[st][:], xs[:, st * 128:(st + 1) * 128],
                            wv_sb[:, dt, :],
                            start=(dt == 0), stop=(dt == ND - 1))
                for st in range(4):
                    if st % 2 == 0:
                        nc.scalar.copy(v_sb[:, sb * 4 + st, :], ps_v[st][:])
                    else:
                        nc.vector.tensor_copy(v_sb[:, sb * 4 + st, :],
                                              ps_v[st][:])

        shared.close()

        # ---- attention + wo ----
        with tc.tile_pool(name="ep", bufs=8) as ep, \
             tc.tile_pool(name="otn", bufs=2) as otn, \
             tc.tile_pool(name="bcp", bufs=2) as bcp, \
             tc.tile_pool(name="wout", bufs=4) as wout, \
             tc.tile_pool(name="ps_s", bufs=3, space="PSUM") as ps_s, \
             tc.tile_pool(name="ps_o", bufs=2, space="PSUM") as ps_o, \
             tc.tile_pool(name="ps_r", bufs=1, space="PSUM") as ps_r, \
             tc.tile_pool(name="ps_w", bufs=2, space="PSUM") as ps_w:

            def wo_block(it):
                for dblk in range(4):
                    pw = ps_w.tile([128, 512], F32, tag="w", name="w")
                    for c in range(4):
                        nc.tensor.matmul(
                            pw[:], prev[0][:, c, it * 128:(it + 1) * 128],
                            wo_sb[:, c, dblk * 512:(dblk + 1) * 512],
                            start=(c == 0), stop=(c == 3))
                    ow = wout.tile([128, 512], F16, tag="ow", name="ow")
                    if dblk % 2 == 0:
                        nc.scalar.copy(ow[:], pw[:])
                    else:
                        nc.vector.tensor_copy(ow[:], pw[:])
                    eng = (nc.sync, nc.scalar)[dblk % 2]
                    eng.dma_start(
                        out_d.ap()[prev[1] + it * 128:prev[1] + (it + 1) * 128,
                                   dblk * 512:(dblk + 1) * 512], ow[:])

            prev = None
            for ib in range(nsb):
                i0 = ib * 512
                nj = 4 * ib + 4
                outn = otn.tile([128, 4, 512], mm_dt, tag="outn", name="outn")
                rsum = bcp.tile([128, 4, 512], F32, tag="rsum", name="rsum")
                rbc = bcp.tile([128, 4, 512], F32, tag="rbc", name="rbc")
                for h in range(4):
                    po = ps_o.tile([128, 512], F32, tag="pv", name="pv")
                    prbc = ps_r.tile([128, 512], F32, tag="rs", name="rs")
                    for tj in range(nj):
                        r = tj - 4 * ib
                        i_lo = 128 * r if r >= 0 else 0
                        pscr = ps_s.tile([128, 512], F32, tag="sc", name="sc")
                        nc.tensor.matmul(
                            pscr[:, i_lo:512], ktr[h][:, tj * 128:(tj + 1) * 128],
                            qtr[h][:, i0 + i_lo:i0 + 512],
                            start=True, stop=True)
                        e_t = ep.tile([128, 512], mm_dt, tag="e", name="e")
                        nc.scalar.activation(e_t[:, i_lo:512], pscr[:, i_lo:512],
                                             AF.Exp)
                        if r >= 0:                # diagonal tile: mask 128 cols
                            nc.vector.tensor_mul(
                                e_t[:, i_lo:i_lo + 128],
                                e_t[:, i_lo:i_lo + 128], tri[:])
                        nc.tensor.matmul(
                            po[:, i_lo:512],
                            v_sb[:, tj, h * 128:(h + 1) * 128], e_t[:, i_lo:512],
                            start=(tj == 0), stop=(tj == nj - 1))
                        nc.tensor.matmul(
                            prbc[:, i_lo:512], ones_sq[:], e_t[:, i_lo:512],
                            start=(tj == 0), stop=(tj == nj - 1))
                    # fast ACT drain frees the single ps_r bank; the slow DVE
                    # reciprocal then runs off the PE-critical path
                    nc.scalar.copy(rsum[:, h, :], prbc[:])
                    nc.vector.reciprocal_approx_fast(rbc[:, h, :], rsum[:, h, :])
                    nc.vector.tensor_mul(outn[:, h, :], po[:], rbc[:, h, :])
                    if prev is not None:
                        wo_block(h)
                prev = (outn, i0)

            for it in range(4):
                wo_block(it)

    nc.compile()
    return nc


def _host_prep(x, freqs_cos, freqs_sin, wq, wk, wv, wo, mm_dt=None, seq=SEQ):
    """Build the 8 per-core input maps with pre-tiled layouts."""
    mm_dt = mm_dt or MM_DT
    npdt = mybir.dt.np(mm_dt)
    bs = x.shape[0]
    nsb = seq // 512
    scale = np.float32(1.0 / math.sqrt(HD))

    rp_npdt = mybir.dt.np(RP_DT)
    cos_e = np.repeat(np.asarray(freqs_cos).T, 2, axis=0).astype(np.float32)
    sin_raw = np.repeat(np.asarray(freqs_sin).T, 2, axis=0).astype(np.float32)
    sin_e = sin_raw.copy()
    sin_e[0::2] = -sin_raw[0::2]      # out[2i] = q[2i]cos - q[2i+1]sin
    trigk = np.ascontiguousarray(np.stack([cos_e, sin_e], axis=1)).astype(rp_npdt)
    trigq = np.ascontiguousarray(
        np.stack([cos_e, sin_e], axis=1) * scale).astype(rp_npdt)

    jr = np.arange(128)[:, None]
    cr = np.arange(128)[None, :]
    tri = (jr <= cr).astype(npdt)     # [j, c]: keep c >= j

    ones_sq = np.ones((128, 128), npdt)

    def wtile(w):  # [DIM, 512] -> [128, 16, 512]
        return np.ascontiguousarray(
            np.asarray(w).reshape(ND, 128, MG).transpose(1, 0, 2)).astype(npdt)

    x_t = []
    for b in range(bs):
        xt = np.asarray(x[b]).reshape(nsb, 512, ND, 128).transpose(0, 3, 2, 1)
        x_t.append(np.ascontiguousarray(xt).astype(npdt))

    in_maps = []
    for core in range(8):
        b, g = divmod(core, 4)
        b = min(b, bs - 1)
        wo_g = np.asarray(wo)[g * MG:(g + 1) * MG, :]
        in_maps.append({
            "x_t": x_t[b],
            "wq_t": wtile(np.asarray(wq)[:, g * MG:(g + 1) * MG]),
            "wk_t": wtile(np.asarray(wk)[:, g * MG:(g + 1) * MG]),
            "wv_t": wtile(np.asarray(wv)[:, g * MG:(g + 1) * MG]),
            "wo_t": np.ascontiguousarray(
                wo_g.reshape(4, 128, DIM).transpose(1, 0, 2)).astype(npdt),
            "trigq": trigq, "trigk": trigk,
            "tri_mask": tri, "ones_sq": ones_sq,
        })
    return in_maps


def kernel(x, freqs_cos, freqs_sin, mask, wq, wk, wv, wo, _trace=False):
    x = np.asarray(x, dtype=np.float32)
    in_maps = _host_prep(x, np.asarray(freqs_cos), np.asarray(freqs_sin),
                         np.asarray(wq), np.asarray(wk), np.asarray(wv),
                         np.asarray(wo))
    if "nc" not in _CACHED:
        _CACHED["nc"] = build_nc()
    nc = _CACHED["nc"]
    res = bass_utils.run_bass_kernel_spmd(nc, in_maps, core_ids=list(range(8)),
                                          trace=_trace)
    if _trace:
        _CACHED["last_exec_time_ns"] = res.exec_time_ns
        _CACHED["last_trace"] = res.instructions_and_trace
    bs = x.shape[0]
    out = np.zeros((bs, SEQ, DIM), dtype=np.float32)
    for core in range(8):
        out[core // 4] += res.results[core]["out"].astype(np.float32)
    return out
